# revision 8
# baseline (speedup 1.0000x reference)
"""Block-circulant matmul kernel for Trainium2 (8 NeuronCores, data-parallel).

Computes out = (x * D) @ M + bias where M is the 4096x4096 block-circulant
matrix built from W[32, 32, 128] (block (i,j) is C_ij[s,t] = W[i,j,(s-t)%128]).

Sharding: batch (4096) split 8 ways -> 512 rows per core; weights replicated.

Two implementations:
 - "fft": 3-stage frequency-domain factorization. Per core: DFT-as-matmul
   (32 mm) -> DVE 32x32 stream-transpose -> per-frequency-slot block-diag
   matmul (32 mm) -> DVE transpose -> iDFT-as-matmul (32 mm). The sigma
   frequency packing puts the 4 real components of a frequency pair-slot c
   at spectrum positions {c, 32+c, 64+c, 96+c} so the quadrant-local DVE
   transpose lands rows exactly where the next stage's matmul needs them.
 - "dense": single big GEMM against the host-materialized circulant matrix.

Everything device-side computes out^T: feature dims on SBUF partitions,
batch on the free dimension.
"""

import os
import numpy as np

import concourse.bass as bass
import concourse.mybir as mybir
from concourse import bacc
from concourse.tile import TileContext
from concourse.bass_utils import run_bass_kernel_spmd

# Problem constants (hardcoded per harness contract).
BATCH = 4096
D_IN = 4096
D_OUT = 4096
BS = 128          # circulant block size
KI = 32           # input blocks
KO = 32           # output blocks
NCORES = 8
BC = BATCH // NCORES      # 512 batch rows per core
NSPLIT = 2                # batch halves per core (pipeline + PSUM sizing)
BH = BC // NSPLIT

IMPL = os.environ.get("BC_IMPL", "fft")
MM_DTYPE = os.environ.get("BC_DTYPE", "fp32")

_NC_CACHE = {}
_PACK_CACHE = {}


def _dt_of(name):
    return {
        "fp32": mybir.dt.float32,
        "f32r": mybir.dt.float32r,
        "bf16": mybir.dt.bfloat16,
    }[name]


# ---------------------------------------------------------------- sigma pack
def _sigma_components():
    """slot c, quadrant Q -> ("re"|"im", f). Pairs (2c+1, 2c+2) for c<31,
    slot 31 holds (63 complex, 0 real, 64 real)."""
    comp = {}
    for c in range(32):
        fa = 2 * c + 1 if c < 31 else 63
        comp[(0, c)] = ("re", fa)
        comp[(1, c)] = ("im", fa)
        if c < 31:
            comp[(2, c)] = ("re", 2 * c + 2)
            comp[(3, c)] = ("im", 2 * c + 2)
        else:
            comp[(2, c)] = ("re", 0)
            comp[(3, c)] = ("re", 64)
    return comp


def _pack_const():
    """Input-independent factor matrices Csig [s, m] and Esig [m, t]."""
    if "const" in _PACK_CACHE:
        return _PACK_CACHE["const"]
    comp = _sigma_components()
    s = np.arange(BS)
    Csig = np.zeros((BS, 128), dtype=np.float64)
    Esig = np.zeros((128, BS), dtype=np.float64)
    for (Q, c), (typ, f) in comp.items():
        m = 32 * Q + c
        ang = 2 * np.pi * f * s / BS
        a = (1.0 if f in (0, 64) else 2.0) / BS
        if typ == "re":
            Csig[:, m] = np.cos(ang)
            Esig[m, :] = a * np.cos(ang)
        else:
            Csig[:, m] = -np.sin(ang)
            Esig[m, :] = -a * np.sin(ang)
    out = (Csig.astype(np.float32), np.ascontiguousarray(Esig.astype(np.float32)))
    _PACK_CACHE["const"] = out
    return out


def _pack_wb(W):
    """Frequency-domain block-diagonal weights WBt [row=(Qr,j), slot, col=(Qc,i)]."""
    comp = _sigma_components()
    Wf = np.fft.fft(W.astype(np.float64), axis=-1)
    Wfr, Wfi = Wf.real, Wf.imag
    WB = np.zeros((32, 128, 128), dtype=np.float64)
    for c in range(32):
        for (qre, qim) in ((0, 1), (2, 3)):
            typ_im = comp[(qim, c)][0]
            f = comp[(qre, c)][1]
            if typ_im == "im":
                wr = Wfr[:, :, f].T  # [j, i]
                wi = Wfi[:, :, f].T
                WB[c, qre*32:(qre+1)*32, qre*32:(qre+1)*32] = wr
                WB[c, qim*32:(qim+1)*32, qre*32:(qre+1)*32] = wi
                WB[c, qre*32:(qre+1)*32, qim*32:(qim+1)*32] = -wi
                WB[c, qim*32:(qim+1)*32, qim*32:(qim+1)*32] = wr
            else:
                f2 = comp[(qim, c)][1]
                WB[c, qre*32:(qre+1)*32, qre*32:(qre+1)*32] = Wfr[:, :, f].T
                WB[c, qim*32:(qim+1)*32, qim*32:(qim+1)*32] = Wfr[:, :, f2].T
    return np.ascontiguousarray(
        WB.transpose(1, 0, 2).astype(np.float32)  # [row, slot, col]
    )


# ---------------------------------------------------------------- fft build
def _build_fft(mm_dtype):
    key = ("fft", mm_dtype)
    if key in _NC_CACHE:
        return _NC_CACHE[key]
    DT = _dt_of(mm_dtype)
    f32 = mybir.dt.float32
    need_round = DT == mybir.dt.float32r
    nsplit = 2 if need_round else 1
    bh = BC // nsplit

    nc = bacc.Bacc(None, target_bir_lowering=False, debug=False)

    xT = nc.dram_tensor("xT", [BS, KI, BC], DT, kind="ExternalInput")
    Csig_d = nc.dram_tensor("Csig", [BS, 128], f32, kind="ExternalInput")
    WBt_d = nc.dram_tensor("WBt", [128, 32, 128], DT, kind="ExternalInput")
    Esig_d = nc.dram_tensor("Esig", [128, BS], DT, kind="ExternalInput")
    Dt_d = nc.dram_tensor("Dt", [BS, KI], f32, kind="ExternalInput")
    bT_d = nc.dram_tensor("bT", [BS, KO], f32, kind="ExternalInput")
    outT = nc.dram_tensor("outT", [KO, BS, BC], f32, kind="ExternalOutput")

    def copy_engine(k):
        return nc.vector if (k % 2 == 0) else None  # None -> scalar ACTIVATE

    def do_copy(k, out, in_):
        if k % 2 == 0:
            nc.vector.tensor_copy(out=out, in_=in_)
        else:
            nc.scalar.activation(
                out=out, in_=in_, func=mybir.ActivationFunctionType.Copy
            )

    with TileContext(nc) as tc:
        with tc.tile_pool(name="consts", bufs=1) as cpool, \
             tc.tile_pool(name="stage", bufs=4) as spool, \
             tc.tile_pool(name="big1", bufs=1) as big1, \
             tc.tile_pool(name="big2", bufs=1) as big2, \
             tc.tile_pool(name="big3", bufs=1) as big3, \
             tc.tile_pool(name="o", bufs=4) as opool, \
             tc.tile_pool(name="psA", bufs=3, space="PSUM") as psA, \
             tc.tile_pool(name="psB", bufs=3, space="PSUM") as psB, \
             tc.tile_pool(name="psC", bufs=2, space="PSUM") as psC:

            csig = cpool.tile([BS, 128], f32)
            esig = cpool.tile([128, BS], DT)
            dt_t = cpool.tile([BS, KI], f32)
            bt_t = cpool.tile([BS, KO], f32)
            wb = cpool.tile([128, 32, 128], DT)
            cd = cpool.tile([BS, KI, 128], DT)
            nc.sync.dma_start(out=csig, in_=Csig_d[:, :])
            nc.sync.dma_start(out=esig, in_=Esig_d[:, :])
            nc.sync.dma_start(out=dt_t, in_=Dt_d[:, :])
            nc.sync.dma_start(out=bt_t, in_=bT_d[:, :])
            nc.sync.dma_start(out=wb, in_=WBt_d[:, :, :])

            # Fold the Bernoulli diagonal into per-j DFT weights.
            for j in range(KI):
                nc.vector.tensor_scalar_mul(
                    out=cd[:, j, :], in0=csig, scalar1=dt_t[:, j : j + 1]
                )

            tdt = f32 if need_round else DT  # transpose-side dtype

            # ---- stage A: spectrum XF[m, b, j] (j innermost for T1 chunks)
            xf = [big1.tile([128, bh, KI], tdt, tag="big1", name=f"xf{h}")
                  for h in range(nsplit)]
            for j in range(KI):
                st = spool.tile([BS, BC], DT, tag="stage")
                nc.sync.dma_start(out=st, in_=xT[:, j, :])
                for h in range(nsplit):
                    ps = psA.tile([128, bh], f32, tag="psA")
                    nc.tensor.matmul(
                        ps, cd[:, j, :], st[:, h * bh : (h + 1) * bh],
                        start=True, stop=True,
                    )
                    do_copy(j + h, xf[h][:, :, j], ps)

            for h in range(nsplit):
                # ---- T1: Z[(Q,j), b, c] = XF[(Q,c), b, j]
                z = big2.tile([128, bh, 32], tdt, tag="big2", name=f"z{h}")
                nc.vector.transpose(out=z, in_=xf[h])
                if need_round:
                    zr = big3.tile([128, bh, 32], DT, tag="big3", name=f"zr{h}")
                    nc.vector.tensor_copy(out=zr, in_=z)
                    z = zr
                # ---- stage B: per-slot block-diagonal frequency matmul
                yz = big1.tile([128, bh, 32], tdt, tag="big1", name=f"yz{h}")
                for c in range(32):
                    ps = psB.tile([128, bh], f32, tag="psB")
                    nc.tensor.matmul(
                        ps, wb[:, c, :], z[:, :, c], start=True, stop=True
                    )
                    do_copy(c, yz[:, :, c], ps)
                # ---- T2: YW[(Q,c), b, i] = YZ[(Q,i), b, c]
                yw = big2.tile([128, bh, 32], tdt, tag="big2", name=f"yw{h}")
                nc.vector.transpose(out=yw, in_=yz)
                if need_round:
                    ywr = big3.tile([128, bh, 32], DT, tag="big3", name=f"ywr{h}")
                    nc.vector.tensor_copy(out=ywr, in_=yw)
                    yw = ywr
                # ---- stage C: iDFT + bias
                for i in range(KO):
                    ps = psC.tile([128, bh], f32, tag="psC")
                    nc.tensor.matmul(
                        ps, esig, yw[:, :, i], start=True, stop=True
                    )
                    oi = opool.tile([BS, bh], f32, tag="o")
                    nc.scalar.activation(
                        out=oi, in_=ps,
                        func=mybir.ActivationFunctionType.Identity,
                        bias=bt_t[:, i : i + 1],
                    )
                    nc.sync.dma_start(
                        out=outT[i, :, h * bh : (h + 1) * bh], in_=oi
                    )

    nc.compile()
    _NC_CACHE[key] = nc
    return nc


def _prep_fft(x, W, D, bias):
    Csig, Esig = _pack_const()
    WBt = _pack_wb(W)
    Dt = np.ascontiguousarray(D.reshape(KI, BS).T)
    bT = np.ascontiguousarray(bias.reshape(KO, BS).T)
    in_maps = []
    for c in range(NCORES):
        xs = x[c * BC : (c + 1) * BC, :]
        xTc = np.ascontiguousarray(xs.reshape(BC, KI, BS).transpose(2, 1, 0))
        in_maps.append(
            {"xT": xTc, "Csig": Csig, "WBt": WBt, "Esig": Esig, "Dt": Dt, "bT": bT}
        )
    return in_maps


# --------------------------------------------------------------- dense build
def _build_dense(mm_dtype):
    key = ("dense", mm_dtype)
    if key in _NC_CACHE:
        return _NC_CACHE[key]
    wdt = _dt_of(mm_dtype)
    f32 = mybir.dt.float32

    nc = bacc.Bacc(None, target_bir_lowering=False, debug=False)

    xT = nc.dram_tensor("xT", [BS, KI, BC], f32, kind="ExternalInput")
    WT = nc.dram_tensor("WT", [KO, BS, KI, BS], wdt, kind="ExternalInput")
    Dt = nc.dram_tensor("Dt", [BS, KI], f32, kind="ExternalInput")
    bT = nc.dram_tensor("bT", [BS, KO], f32, kind="ExternalInput")
    outT = nc.dram_tensor("outT", [KO, BS, BC], f32, kind="ExternalOutput")

    xd_dt = f32 if mm_dtype == "fp32" else wdt

    with TileContext(nc) as tc:
        with tc.tile_pool(name="consts", bufs=1) as cpool, \
             tc.tile_pool(name="stage", bufs=6) as spool, \
             tc.tile_pool(name="xd", bufs=1) as xdpool, \
             tc.tile_pool(name="w", bufs=3) as wpool, \
             tc.tile_pool(name="o", bufs=3) as opool, \
             tc.tile_pool(name="ps", bufs=4, space="PSUM") as pspool:

            dt_tile = cpool.tile([BS, KI], f32)
            bt_tile = cpool.tile([BS, KO], f32)
            nc.sync.dma_start(out=dt_tile, in_=Dt[:, :])
            nc.sync.dma_start(out=bt_tile, in_=bT[:, :])

            xd = xdpool.tile([BS, KI, BC], xd_dt)
            for j in range(KI):
                st = spool.tile([BS, BC], f32, tag="stage")
                nc.sync.dma_start(out=st, in_=xT[:, j, :])
                nc.vector.tensor_scalar_mul(
                    out=xd[:, j, :], in0=st, scalar1=dt_tile[:, j : j + 1]
                )

            for i in range(KO):
                wi = wpool.tile([BS, KI, BS], wdt, tag="w")
                nc.sync.dma_start(out=wi, in_=WT[i])
                ps = pspool.tile([BS, BC], f32, tag="ps")
                for j in range(KI):
                    nc.tensor.matmul(
                        ps, wi[:, j, :], xd[:, j, :],
                        start=(j == 0), stop=(j == KI - 1),
                    )
                oi = opool.tile([BS, BC], f32, tag="o")
                nc.vector.tensor_scalar_add(
                    out=oi, in0=ps, scalar1=bt_tile[:, i : i + 1]
                )
                nc.sync.dma_start(out=outT[i], in_=oi)

    nc.compile()
    _NC_CACHE[key] = nc
    return nc


def _prep_dense(x, W, D, bias, mm_dtype):
    s = np.arange(BS)
    roll = (s[:, None] - s[None, :]) % BS
    M4 = W[:, :, roll]                                   # [i, j, s, t]
    WT = np.ascontiguousarray(M4.transpose(0, 2, 1, 3))  # [i, s, j, t]
    if mm_dtype == "bf16":
        import ml_dtypes
        WT = WT.astype(ml_dtypes.bfloat16)
    Dt = np.ascontiguousarray(D.reshape(KI, BS).T)
    bT = np.ascontiguousarray(bias.reshape(KO, BS).T)
    in_maps = []
    for c in range(NCORES):
        xs = x[c * BC : (c + 1) * BC, :]
        xTc = np.ascontiguousarray(xs.reshape(BC, KI, BS).transpose(2, 1, 0))
        in_maps.append({"xT": xTc, "WT": WT, "Dt": Dt, "bT": bT})
    return in_maps


# ------------------------------------------------------------------- driver
def _run(inputs, trace=False):
    x = np.asarray(inputs["x"], dtype=np.float32)
    W = np.asarray(inputs["W"], dtype=np.float32)
    D = np.asarray(inputs["D_bernoulli"], dtype=np.float32)
    bias = np.asarray(inputs["bias"], dtype=np.float32)

    if IMPL == "fft":
        nc = _build_fft(MM_DTYPE)
        in_maps = _prep_fft(x, W, D, bias)
    else:
        nc = _build_dense(MM_DTYPE)
        in_maps = _prep_dense(x, W, D, bias, MM_DTYPE)

    res = run_bass_kernel_spmd(nc, in_maps, list(range(NCORES)), trace=trace)
    out = np.empty((BATCH, D_OUT), dtype=np.float32)
    for c in range(NCORES):
        oT = res.results[c]["outT"]                  # [i, t, b]
        out[c * BC : (c + 1) * BC, :] = oT.transpose(2, 0, 1).reshape(BC, D_OUT)
    return out, res


def kernel(**inputs) -> np.ndarray:
    out, _ = _run(inputs, trace=False)
    return out


# revision 9
# speedup vs baseline: 1.0169x; 1.0169x over previous
"""Block-circulant matmul kernel for Trainium2 (8 NeuronCores, data-parallel).

Computes out = (x * D) @ M + bias where M is the 4096x4096 block-circulant
matrix built from W[32, 32, 128] (block (i,j) is C_ij[s,t] = W[i,j,(s-t)%128]).

Sharding: batch (4096) split 8 ways -> 512 rows per core; weights replicated.

Two implementations:
 - "fft": 3-stage frequency-domain factorization. Per core: DFT-as-matmul
   (32 mm) -> DVE 32x32 stream-transpose -> per-frequency-slot block-diag
   matmul (32 mm) -> DVE transpose -> iDFT-as-matmul (32 mm). The sigma
   frequency packing puts the 4 real components of a frequency pair-slot c
   at spectrum positions {c, 32+c, 64+c, 96+c} so the quadrant-local DVE
   transpose lands rows exactly where the next stage's matmul needs them.
 - "dense": single big GEMM against the host-materialized circulant matrix.

Everything device-side computes out^T: feature dims on SBUF partitions,
batch on the free dimension.
"""

import os
import numpy as np

import concourse.bass as bass
import concourse.mybir as mybir
from concourse import bacc
from concourse.tile import TileContext
from concourse.bass_utils import run_bass_kernel_spmd

# Problem constants (hardcoded per harness contract).
BATCH = 4096
D_IN = 4096
D_OUT = 4096
BS = 128          # circulant block size
KI = 32           # input blocks
KO = 32           # output blocks
NCORES = 8
BC = BATCH // NCORES      # 512 batch rows per core
NSPLIT = 2                # batch halves per core (pipeline + PSUM sizing)
BH = BC // NSPLIT

IMPL = os.environ.get("BC_IMPL", "fft")
MM_DTYPE = os.environ.get("BC_DTYPE", "fp32")

_NC_CACHE = {}
_PACK_CACHE = {}


def _dt_of(name):
    return {
        "fp32": mybir.dt.float32,
        "f32r": mybir.dt.float32r,
        "bf16": mybir.dt.bfloat16,
    }[name]


# ---------------------------------------------------------------- sigma pack
def _sigma_components():
    """slot c, quadrant Q -> ("re"|"im", f). Pairs (2c+1, 2c+2) for c<31,
    slot 31 holds (63 complex, 0 real, 64 real)."""
    comp = {}
    for c in range(32):
        fa = 2 * c + 1 if c < 31 else 63
        comp[(0, c)] = ("re", fa)
        comp[(1, c)] = ("im", fa)
        if c < 31:
            comp[(2, c)] = ("re", 2 * c + 2)
            comp[(3, c)] = ("im", 2 * c + 2)
        else:
            comp[(2, c)] = ("re", 0)
            comp[(3, c)] = ("re", 64)
    return comp


def _pack_const():
    """Input-independent factor matrices Csig [s, m] and Esig [m, t]."""
    if "const" in _PACK_CACHE:
        return _PACK_CACHE["const"]
    comp = _sigma_components()
    s = np.arange(BS)
    Csig = np.zeros((BS, 128), dtype=np.float64)
    Esig = np.zeros((128, BS), dtype=np.float64)
    for (Q, c), (typ, f) in comp.items():
        m = 32 * Q + c
        ang = 2 * np.pi * f * s / BS
        a = (1.0 if f in (0, 64) else 2.0) / BS
        if typ == "re":
            Csig[:, m] = np.cos(ang)
            Esig[m, :] = a * np.cos(ang)
        else:
            Csig[:, m] = -np.sin(ang)
            Esig[m, :] = -a * np.sin(ang)
    out = (Csig.astype(np.float32), np.ascontiguousarray(Esig.astype(np.float32)))
    _PACK_CACHE["const"] = out
    return out


def _pack_wb(W):
    """Frequency-domain block-diagonal weights WBt [row=(Qr,j), slot, col=(Qc,i)]."""
    comp = _sigma_components()
    Wf = np.fft.fft(W.astype(np.float64), axis=-1)
    Wfr, Wfi = Wf.real, Wf.imag
    WB = np.zeros((32, 128, 128), dtype=np.float64)
    for c in range(32):
        for (qre, qim) in ((0, 1), (2, 3)):
            typ_im = comp[(qim, c)][0]
            f = comp[(qre, c)][1]
            if typ_im == "im":
                wr = Wfr[:, :, f].T  # [j, i]
                wi = Wfi[:, :, f].T
                WB[c, qre*32:(qre+1)*32, qre*32:(qre+1)*32] = wr
                WB[c, qim*32:(qim+1)*32, qre*32:(qre+1)*32] = wi
                WB[c, qre*32:(qre+1)*32, qim*32:(qim+1)*32] = -wi
                WB[c, qim*32:(qim+1)*32, qim*32:(qim+1)*32] = wr
            else:
                f2 = comp[(qim, c)][1]
                WB[c, qre*32:(qre+1)*32, qre*32:(qre+1)*32] = Wfr[:, :, f].T
                WB[c, qim*32:(qim+1)*32, qim*32:(qim+1)*32] = Wfr[:, :, f2].T
    return np.ascontiguousarray(
        WB.transpose(1, 0, 2).astype(np.float32)  # [row, slot, col]
    )


# ---------------------------------------------------------------- fft build
def _build_fft(mm_dtype):
    key = ("fft", mm_dtype)
    if key in _NC_CACHE:
        return _NC_CACHE[key]
    DT = _dt_of(mm_dtype)
    f32 = mybir.dt.float32
    need_round = DT == mybir.dt.float32r
    nsplit = 2 if need_round else 1
    bh = BC // nsplit

    nc = bacc.Bacc(None, target_bir_lowering=False, debug=False)

    xT = nc.dram_tensor("xT", [BS, KI, BC], DT, kind="ExternalInput")
    Csig_d = nc.dram_tensor("Csig", [BS, 128], f32, kind="ExternalInput")
    WBt_d = nc.dram_tensor("WBt", [128, 32, 128], DT, kind="ExternalInput")
    Esig_d = nc.dram_tensor("Esig", [128, BS], DT, kind="ExternalInput")
    Dt_d = nc.dram_tensor("Dt", [BS, KI], f32, kind="ExternalInput")
    bT_d = nc.dram_tensor("bT", [BS, KO], f32, kind="ExternalInput")
    outT = nc.dram_tensor("outT", [KO, BS, BC], f32, kind="ExternalOutput")

    def copy_engine(k):
        return nc.vector if (k % 2 == 0) else None  # None -> scalar ACTIVATE

    def do_copy(k, out, in_):
        if k % 2 == 0:
            nc.vector.tensor_copy(out=out, in_=in_)
        else:
            nc.scalar.activation(
                out=out, in_=in_, func=mybir.ActivationFunctionType.Copy
            )

    with TileContext(nc) as tc:
        with tc.tile_pool(name="consts", bufs=1) as cpool, \
             tc.tile_pool(name="stage", bufs=4) as spool, \
             tc.tile_pool(name="big1", bufs=1) as big1, \
             tc.tile_pool(name="big2", bufs=1) as big2, \
             tc.tile_pool(name="big3", bufs=1) as big3, \
             tc.tile_pool(name="o", bufs=4) as opool, \
             tc.tile_pool(name="psAll", bufs=8, space="PSUM") as psAll:

            psA = psB = psC = psAll
            csig = cpool.tile([BS, 128], f32)
            esig = cpool.tile([128, BS], DT)
            dt_t = cpool.tile([BS, KI], f32)
            bt_t = cpool.tile([BS, KO], f32)
            wb = cpool.tile([128, 32, 128], DT)
            cd = cpool.tile([BS, KI, 128], DT)
            nc.sync.dma_start(out=csig, in_=Csig_d[:, :])
            nc.sync.dma_start(out=esig, in_=Esig_d[:, :])
            nc.sync.dma_start(out=dt_t, in_=Dt_d[:, :])
            nc.sync.dma_start(out=bt_t, in_=bT_d[:, :])
            nc.sync.dma_start(out=wb, in_=WBt_d[:, :, :])

            # Fold the Bernoulli diagonal into per-j DFT weights.
            for j in range(KI):
                nc.vector.tensor_scalar_mul(
                    out=cd[:, j, :], in0=csig, scalar1=dt_t[:, j : j + 1]
                )

            tdt = f32 if need_round else DT  # transpose-side dtype

            # ---- stage A: spectrum XF[m, b, j] (j innermost for T1 chunks)
            xf = [big1.tile([128, bh, KI], tdt, tag="big1", name=f"xf{h}")
                  for h in range(nsplit)]
            for j in range(KI):
                st = spool.tile([BS, BC], DT, tag="stage")
                nc.sync.dma_start(out=st, in_=xT[:, j, :])
                for h in range(nsplit):
                    ps = psA.tile([128, bh], f32, tag="ps", name=f"psa{j}_{h}")
                    nc.tensor.matmul(
                        ps, cd[:, j, :], st[:, h * bh : (h + 1) * bh],
                        start=True, stop=True,
                    )
                    do_copy(j + h, xf[h][:, :, j], ps)

            for h in range(nsplit):
                # ---- T1: Z[(Q,j), b, c] = XF[(Q,c), b, j]
                z = big2.tile([128, bh, 32], tdt, tag="big2", name=f"z{h}")
                nc.vector.transpose(out=z, in_=xf[h])
                if need_round:
                    zr = big3.tile([128, bh, 32], DT, tag="big3", name=f"zr{h}")
                    nc.vector.tensor_copy(out=zr, in_=z)
                    z = zr
                # ---- stage B: per-slot block-diagonal frequency matmul
                yz = big1.tile([128, bh, 32], tdt, tag="big1", name=f"yz{h}")
                for c in range(32):
                    ps = psB.tile([128, bh], f32, tag="ps", name=f"psb{c}_{h}")
                    nc.tensor.matmul(
                        ps, wb[:, c, :], z[:, :, c], start=True, stop=True
                    )
                    do_copy(c, yz[:, :, c], ps)
                # ---- T2: YW[(Q,c), b, i] = YZ[(Q,i), b, c]
                yw = big2.tile([128, bh, 32], tdt, tag="big2", name=f"yw{h}")
                nc.vector.transpose(out=yw, in_=yz)
                if need_round:
                    ywr = big3.tile([128, bh, 32], DT, tag="big3", name=f"ywr{h}")
                    nc.vector.tensor_copy(out=ywr, in_=yw)
                    yw = ywr
                # ---- stage C: iDFT + bias
                for i in range(KO):
                    ps = psC.tile([128, bh], f32, tag="ps", name=f"psc{i}_{h}")
                    nc.tensor.matmul(
                        ps, esig, yw[:, :, i], start=True, stop=True
                    )
                    oi = opool.tile([BS, bh], f32, tag="o")
                    nc.scalar.activation(
                        out=oi, in_=ps,
                        func=mybir.ActivationFunctionType.Identity,
                        bias=bt_t[:, i : i + 1],
                    )
                    nc.sync.dma_start(
                        out=outT[i, :, h * bh : (h + 1) * bh], in_=oi
                    )

    nc.compile()
    _NC_CACHE[key] = nc
    return nc


def _prep_fft(x, W, D, bias):
    Csig, Esig = _pack_const()
    WBt = _pack_wb(W)
    Dt = np.ascontiguousarray(D.reshape(KI, BS).T)
    bT = np.ascontiguousarray(bias.reshape(KO, BS).T)
    in_maps = []
    for c in range(NCORES):
        xs = x[c * BC : (c + 1) * BC, :]
        xTc = np.ascontiguousarray(xs.reshape(BC, KI, BS).transpose(2, 1, 0))
        in_maps.append(
            {"xT": xTc, "Csig": Csig, "WBt": WBt, "Esig": Esig, "Dt": Dt, "bT": bT}
        )
    return in_maps


# --------------------------------------------------------------- dense build
def _build_dense(mm_dtype):
    key = ("dense", mm_dtype)
    if key in _NC_CACHE:
        return _NC_CACHE[key]
    wdt = _dt_of(mm_dtype)
    f32 = mybir.dt.float32

    nc = bacc.Bacc(None, target_bir_lowering=False, debug=False)

    xT = nc.dram_tensor("xT", [BS, KI, BC], f32, kind="ExternalInput")
    WT = nc.dram_tensor("WT", [KO, BS, KI, BS], wdt, kind="ExternalInput")
    Dt = nc.dram_tensor("Dt", [BS, KI], f32, kind="ExternalInput")
    bT = nc.dram_tensor("bT", [BS, KO], f32, kind="ExternalInput")
    outT = nc.dram_tensor("outT", [KO, BS, BC], f32, kind="ExternalOutput")

    xd_dt = f32 if mm_dtype == "fp32" else wdt

    with TileContext(nc) as tc:
        with tc.tile_pool(name="consts", bufs=1) as cpool, \
             tc.tile_pool(name="stage", bufs=6) as spool, \
             tc.tile_pool(name="xd", bufs=1) as xdpool, \
             tc.tile_pool(name="w", bufs=3) as wpool, \
             tc.tile_pool(name="o", bufs=3) as opool, \
             tc.tile_pool(name="ps", bufs=4, space="PSUM") as pspool:

            dt_tile = cpool.tile([BS, KI], f32)
            bt_tile = cpool.tile([BS, KO], f32)
            nc.sync.dma_start(out=dt_tile, in_=Dt[:, :])
            nc.sync.dma_start(out=bt_tile, in_=bT[:, :])

            xd = xdpool.tile([BS, KI, BC], xd_dt)
            for j in range(KI):
                st = spool.tile([BS, BC], f32, tag="stage")
                nc.sync.dma_start(out=st, in_=xT[:, j, :])
                nc.vector.tensor_scalar_mul(
                    out=xd[:, j, :], in0=st, scalar1=dt_tile[:, j : j + 1]
                )

            for i in range(KO):
                wi = wpool.tile([BS, KI, BS], wdt, tag="w")
                nc.sync.dma_start(out=wi, in_=WT[i])
                ps = pspool.tile([BS, BC], f32, tag="ps")
                for j in range(KI):
                    nc.tensor.matmul(
                        ps, wi[:, j, :], xd[:, j, :],
                        start=(j == 0), stop=(j == KI - 1),
                    )
                oi = opool.tile([BS, BC], f32, tag="o")
                nc.vector.tensor_scalar_add(
                    out=oi, in0=ps, scalar1=bt_tile[:, i : i + 1]
                )
                nc.sync.dma_start(out=outT[i], in_=oi)

    nc.compile()
    _NC_CACHE[key] = nc
    return nc


def _prep_dense(x, W, D, bias, mm_dtype):
    s = np.arange(BS)
    roll = (s[:, None] - s[None, :]) % BS
    M4 = W[:, :, roll]                                   # [i, j, s, t]
    WT = np.ascontiguousarray(M4.transpose(0, 2, 1, 3))  # [i, s, j, t]
    if mm_dtype == "bf16":
        import ml_dtypes
        WT = WT.astype(ml_dtypes.bfloat16)
    Dt = np.ascontiguousarray(D.reshape(KI, BS).T)
    bT = np.ascontiguousarray(bias.reshape(KO, BS).T)
    in_maps = []
    for c in range(NCORES):
        xs = x[c * BC : (c + 1) * BC, :]
        xTc = np.ascontiguousarray(xs.reshape(BC, KI, BS).transpose(2, 1, 0))
        in_maps.append({"xT": xTc, "WT": WT, "Dt": Dt, "bT": bT})
    return in_maps


# ------------------------------------------------------------------- driver
def _run(inputs, trace=False):
    x = np.asarray(inputs["x"], dtype=np.float32)
    W = np.asarray(inputs["W"], dtype=np.float32)
    D = np.asarray(inputs["D_bernoulli"], dtype=np.float32)
    bias = np.asarray(inputs["bias"], dtype=np.float32)

    if IMPL == "fft":
        nc = _build_fft(MM_DTYPE)
        in_maps = _prep_fft(x, W, D, bias)
    else:
        nc = _build_dense(MM_DTYPE)
        in_maps = _prep_dense(x, W, D, bias, MM_DTYPE)

    res = run_bass_kernel_spmd(nc, in_maps, list(range(NCORES)), trace=trace)
    out = np.empty((BATCH, D_OUT), dtype=np.float32)
    for c in range(NCORES):
        oT = res.results[c]["outT"]                  # [i, t, b]
        out[c * BC : (c + 1) * BC, :] = oT.transpose(2, 0, 1).reshape(BC, D_OUT)
    return out, res


def kernel(**inputs) -> np.ndarray:
    out, _ = _run(inputs, trace=False)
    return out


# revision 10
# speedup vs baseline: 1.0900x; 1.0719x over previous
"""Block-circulant matmul kernel for Trainium2 (8 NeuronCores, data-parallel).

Computes out = (x * D) @ M + bias where M is the 4096x4096 block-circulant
matrix built from W[32, 32, 128] (block (i,j) is C_ij[s,t] = W[i,j,(s-t)%128]).

Sharding: batch (4096) split 8 ways -> 512 rows per core; weights replicated.

Two implementations:
 - "fft": 3-stage frequency-domain factorization. Per core: DFT-as-matmul
   (32 mm) -> DVE 32x32 stream-transpose -> per-frequency-slot block-diag
   matmul (32 mm) -> DVE transpose -> iDFT-as-matmul (32 mm). The sigma
   frequency packing puts the 4 real components of a frequency pair-slot c
   at spectrum positions {c, 32+c, 64+c, 96+c} so the quadrant-local DVE
   transpose lands rows exactly where the next stage's matmul needs them.
 - "dense": single big GEMM against the host-materialized circulant matrix.

Everything device-side computes out^T: feature dims on SBUF partitions,
batch on the free dimension.
"""

import os
import numpy as np

import concourse.bass as bass
import concourse.mybir as mybir
from concourse import bacc
from concourse.tile import TileContext
from concourse.bass_utils import run_bass_kernel_spmd

# Problem constants (hardcoded per harness contract).
BATCH = 4096
D_IN = 4096
D_OUT = 4096
BS = 128          # circulant block size
KI = 32           # input blocks
KO = 32           # output blocks
NCORES = 8
BC = BATCH // NCORES      # 512 batch rows per core
NSPLIT = 2                # batch halves per core (pipeline + PSUM sizing)
BH = BC // NSPLIT

IMPL = os.environ.get("BC_IMPL", "fft")
MM_DTYPE = os.environ.get("BC_DTYPE", "fp32")

_NC_CACHE = {}
_PACK_CACHE = {}


def _dt_of(name):
    return {
        "fp32": mybir.dt.float32,
        "f32r": mybir.dt.float32r,
        "bf16": mybir.dt.bfloat16,
    }[name]


# ---------------------------------------------------------------- sigma pack
def _sigma_components():
    """slot c, quadrant Q -> ("re"|"im", f). Pairs (2c+1, 2c+2) for c<31,
    slot 31 holds (63 complex, 0 real, 64 real)."""
    comp = {}
    for c in range(32):
        fa = 2 * c + 1 if c < 31 else 63
        comp[(0, c)] = ("re", fa)
        comp[(1, c)] = ("im", fa)
        if c < 31:
            comp[(2, c)] = ("re", 2 * c + 2)
            comp[(3, c)] = ("im", 2 * c + 2)
        else:
            comp[(2, c)] = ("re", 0)
            comp[(3, c)] = ("re", 64)
    return comp


def _pack_const():
    """Input-independent factor matrices Csig [s, m] and Esig [m, t]."""
    if "const" in _PACK_CACHE:
        return _PACK_CACHE["const"]
    comp = _sigma_components()
    s = np.arange(BS)
    Csig = np.zeros((BS, 128), dtype=np.float64)
    Esig = np.zeros((128, BS), dtype=np.float64)
    for (Q, c), (typ, f) in comp.items():
        m = 32 * Q + c
        ang = 2 * np.pi * f * s / BS
        a = (1.0 if f in (0, 64) else 2.0) / BS
        if typ == "re":
            Csig[:, m] = np.cos(ang)
            Esig[m, :] = a * np.cos(ang)
        else:
            Csig[:, m] = -np.sin(ang)
            Esig[m, :] = -a * np.sin(ang)
    out = (Csig.astype(np.float32), np.ascontiguousarray(Esig.astype(np.float32)))
    _PACK_CACHE["const"] = out
    return out


def _pack_wb(W):
    """Frequency-domain block-diagonal weights WBt [row=(Qr,j), slot, col=(Qc,i)]."""
    comp = _sigma_components()
    Wf = np.fft.fft(W.astype(np.float64), axis=-1)
    Wfr, Wfi = Wf.real, Wf.imag
    WB = np.zeros((32, 128, 128), dtype=np.float64)
    for c in range(32):
        for (qre, qim) in ((0, 1), (2, 3)):
            typ_im = comp[(qim, c)][0]
            f = comp[(qre, c)][1]
            if typ_im == "im":
                wr = Wfr[:, :, f].T  # [j, i]
                wi = Wfi[:, :, f].T
                WB[c, qre*32:(qre+1)*32, qre*32:(qre+1)*32] = wr
                WB[c, qim*32:(qim+1)*32, qre*32:(qre+1)*32] = wi
                WB[c, qre*32:(qre+1)*32, qim*32:(qim+1)*32] = -wi
                WB[c, qim*32:(qim+1)*32, qim*32:(qim+1)*32] = wr
            else:
                f2 = comp[(qim, c)][1]
                WB[c, qre*32:(qre+1)*32, qre*32:(qre+1)*32] = Wfr[:, :, f].T
                WB[c, qim*32:(qim+1)*32, qim*32:(qim+1)*32] = Wfr[:, :, f2].T
    return np.ascontiguousarray(
        WB.transpose(1, 0, 2).astype(np.float32)  # [row, slot, col]
    )


# ---------------------------------------------------------------- fft build
def _build_fft(mm_dtype):
    key = ("fft", mm_dtype)
    if key in _NC_CACHE:
        return _NC_CACHE[key]
    DT = _dt_of(mm_dtype)
    f32 = mybir.dt.float32
    need_round = DT == mybir.dt.float32r
    nsplit = 2 if need_round else 1
    bh = BC // nsplit

    nc = bacc.Bacc(None, target_bir_lowering=False, debug=False)

    xT = nc.dram_tensor("xT", [BS, KI, BC], DT, kind="ExternalInput")
    Csig_d = nc.dram_tensor("Csig", [BS, 128], f32, kind="ExternalInput")
    WBt_d = nc.dram_tensor("WBt", [128, 32, 128], DT, kind="ExternalInput")
    Esig_d = nc.dram_tensor("Esig", [128, BS], DT, kind="ExternalInput")
    Dt_d = nc.dram_tensor("Dt", [BS, KI], f32, kind="ExternalInput")
    bT_d = nc.dram_tensor("bT", [BS, KO], f32, kind="ExternalInput")
    outT = nc.dram_tensor("outT", [KO, BS, BC], f32, kind="ExternalOutput")

    def copy_engine(k):
        return nc.vector if (k % 2 == 0) else None  # None -> scalar ACTIVATE

    def do_copy(k, out, in_):
        if k % 2 == 0:
            nc.vector.tensor_copy(out=out, in_=in_)
        else:
            nc.scalar.activation(
                out=out, in_=in_, func=mybir.ActivationFunctionType.Copy
            )

    with TileContext(nc) as tc:
        with tc.tile_pool(name="consts", bufs=1) as cpool, \
             tc.tile_pool(name="stage", bufs=4) as spool, \
             tc.tile_pool(name="big1", bufs=1) as big1, \
             tc.tile_pool(name="big2", bufs=1) as big2, \
             tc.tile_pool(name="big3", bufs=1) as big3, \
             tc.tile_pool(name="o", bufs=4) as opool, \
             tc.tile_pool(name="psAll", bufs=8, space="PSUM") as psAll:

            psA = psB = psC = psAll
            csig = cpool.tile([BS, 128], f32)
            esig = cpool.tile([128, BS], DT)
            dt_t = cpool.tile([BS, KI], f32)
            bt_t = cpool.tile([BS, KO], f32)
            wb = cpool.tile([128, 32, 128], DT)
            cd = cpool.tile([BS, KI, 128], DT)
            nc.sync.dma_start(out=csig, in_=Csig_d[:, :])
            nc.sync.dma_start(out=esig, in_=Esig_d[:, :])
            nc.sync.dma_start(out=dt_t, in_=Dt_d[:, :])
            nc.sync.dma_start(out=bt_t, in_=bT_d[:, :])
            nc.sync.dma_start(out=wb, in_=WBt_d[:, :, :])

            # Fold the Bernoulli diagonal into per-j DFT weights.
            for j in range(KI):
                nc.vector.tensor_scalar_mul(
                    out=cd[:, j, :], in0=csig, scalar1=dt_t[:, j : j + 1]
                )

            tdt = f32 if need_round else DT  # transpose-side dtype

            # ---- stage A: spectrum XF[m, b, j] (j innermost for T1 chunks)
            xf = [big1.tile([128, bh, KI], tdt, tag="big1", name=f"xf{h}")
                  for h in range(nsplit)]
            for j in range(KI):
                st = spool.tile([BS, BC], DT, tag="stage")
                nc.sync.dma_start(out=st, in_=xT[:, j, :])
                for h in range(nsplit):
                    ps = psA.tile([128, bh], f32, tag="ps", name=f"psa{j}_{h}")
                    nc.tensor.matmul(
                        ps, cd[:, j, :], st[:, h * bh : (h + 1) * bh],
                        start=True, stop=True,
                    )
                    do_copy(j + h, xf[h][:, :, j], ps)

            for h in range(nsplit):
                # ---- T1: Z[(Q,j), b, c] = XF[(Q,c), b, j]
                z = big2.tile([128, 32, bh], tdt, tag="big2", name=f"z{h}")
                nc.vector.transpose(out=z.transpose([0, 2, 1]), in_=xf[h])
                if need_round:
                    zr = big3.tile([128, 32, bh], DT, tag="big3", name=f"zr{h}")
                    nc.vector.tensor_copy(out=zr, in_=z)
                    z = zr
                # ---- stage B: per-slot block-diagonal frequency matmul
                yz = big1.tile([128, bh, 32], tdt, tag="big1", name=f"yz{h}")
                for c in range(32):
                    ps = psB.tile([128, bh], f32, tag="ps", name=f"psb{c}_{h}")
                    nc.tensor.matmul(
                        ps, wb[:, c, :], z[:, c, :], start=True, stop=True
                    )
                    do_copy(c, yz[:, :, c], ps)
                # ---- T2: YW[(Q,c), b, i] = YZ[(Q,i), b, c]
                yw = big2.tile([128, 32, bh], tdt, tag="big2", name=f"yw{h}")
                nc.vector.transpose(out=yw.transpose([0, 2, 1]), in_=yz)
                if need_round:
                    ywr = big3.tile([128, 32, bh], DT, tag="big3", name=f"ywr{h}")
                    nc.vector.tensor_copy(out=ywr, in_=yw)
                    yw = ywr
                # ---- stage C: iDFT + bias
                for i in range(KO):
                    ps = psC.tile([128, bh], f32, tag="ps", name=f"psc{i}_{h}")
                    nc.tensor.matmul(
                        ps, esig, yw[:, i, :], start=True, stop=True
                    )
                    oi = opool.tile([BS, bh], f32, tag="o")
                    nc.scalar.activation(
                        out=oi, in_=ps,
                        func=mybir.ActivationFunctionType.Identity,
                        bias=bt_t[:, i : i + 1],
                    )
                    nc.sync.dma_start(
                        out=outT[i, :, h * bh : (h + 1) * bh], in_=oi
                    )

    nc.compile()
    _NC_CACHE[key] = nc
    return nc


def _prep_fft(x, W, D, bias):
    Csig, Esig = _pack_const()
    WBt = _pack_wb(W)
    Dt = np.ascontiguousarray(D.reshape(KI, BS).T)
    bT = np.ascontiguousarray(bias.reshape(KO, BS).T)
    in_maps = []
    for c in range(NCORES):
        xs = x[c * BC : (c + 1) * BC, :]
        xTc = np.ascontiguousarray(xs.reshape(BC, KI, BS).transpose(2, 1, 0))
        in_maps.append(
            {"xT": xTc, "Csig": Csig, "WBt": WBt, "Esig": Esig, "Dt": Dt, "bT": bT}
        )
    return in_maps


# --------------------------------------------------------------- dense build
def _build_dense(mm_dtype):
    key = ("dense", mm_dtype)
    if key in _NC_CACHE:
        return _NC_CACHE[key]
    wdt = _dt_of(mm_dtype)
    f32 = mybir.dt.float32

    nc = bacc.Bacc(None, target_bir_lowering=False, debug=False)

    xT = nc.dram_tensor("xT", [BS, KI, BC], f32, kind="ExternalInput")
    WT = nc.dram_tensor("WT", [KO, BS, KI, BS], wdt, kind="ExternalInput")
    Dt = nc.dram_tensor("Dt", [BS, KI], f32, kind="ExternalInput")
    bT = nc.dram_tensor("bT", [BS, KO], f32, kind="ExternalInput")
    outT = nc.dram_tensor("outT", [KO, BS, BC], f32, kind="ExternalOutput")

    xd_dt = f32 if mm_dtype == "fp32" else wdt

    with TileContext(nc) as tc:
        with tc.tile_pool(name="consts", bufs=1) as cpool, \
             tc.tile_pool(name="stage", bufs=6) as spool, \
             tc.tile_pool(name="xd", bufs=1) as xdpool, \
             tc.tile_pool(name="w", bufs=3) as wpool, \
             tc.tile_pool(name="o", bufs=3) as opool, \
             tc.tile_pool(name="ps", bufs=4, space="PSUM") as pspool:

            dt_tile = cpool.tile([BS, KI], f32)
            bt_tile = cpool.tile([BS, KO], f32)
            nc.sync.dma_start(out=dt_tile, in_=Dt[:, :])
            nc.sync.dma_start(out=bt_tile, in_=bT[:, :])

            xd = xdpool.tile([BS, KI, BC], xd_dt)
            for j in range(KI):
                st = spool.tile([BS, BC], f32, tag="stage")
                nc.sync.dma_start(out=st, in_=xT[:, j, :])
                nc.vector.tensor_scalar_mul(
                    out=xd[:, j, :], in0=st, scalar1=dt_tile[:, j : j + 1]
                )

            for i in range(KO):
                wi = wpool.tile([BS, KI, BS], wdt, tag="w")
                nc.sync.dma_start(out=wi, in_=WT[i])
                ps = pspool.tile([BS, BC], f32, tag="ps")
                for j in range(KI):
                    nc.tensor.matmul(
                        ps, wi[:, j, :], xd[:, j, :],
                        start=(j == 0), stop=(j == KI - 1),
                    )
                oi = opool.tile([BS, BC], f32, tag="o")
                nc.vector.tensor_scalar_add(
                    out=oi, in0=ps, scalar1=bt_tile[:, i : i + 1]
                )
                nc.sync.dma_start(out=outT[i], in_=oi)

    nc.compile()
    _NC_CACHE[key] = nc
    return nc


def _prep_dense(x, W, D, bias, mm_dtype):
    s = np.arange(BS)
    roll = (s[:, None] - s[None, :]) % BS
    M4 = W[:, :, roll]                                   # [i, j, s, t]
    WT = np.ascontiguousarray(M4.transpose(0, 2, 1, 3))  # [i, s, j, t]
    if mm_dtype == "bf16":
        import ml_dtypes
        WT = WT.astype(ml_dtypes.bfloat16)
    Dt = np.ascontiguousarray(D.reshape(KI, BS).T)
    bT = np.ascontiguousarray(bias.reshape(KO, BS).T)
    in_maps = []
    for c in range(NCORES):
        xs = x[c * BC : (c + 1) * BC, :]
        xTc = np.ascontiguousarray(xs.reshape(BC, KI, BS).transpose(2, 1, 0))
        in_maps.append({"xT": xTc, "WT": WT, "Dt": Dt, "bT": bT})
    return in_maps


# ------------------------------------------------------------------- driver
def _run(inputs, trace=False):
    x = np.asarray(inputs["x"], dtype=np.float32)
    W = np.asarray(inputs["W"], dtype=np.float32)
    D = np.asarray(inputs["D_bernoulli"], dtype=np.float32)
    bias = np.asarray(inputs["bias"], dtype=np.float32)

    if IMPL == "fft":
        nc = _build_fft(MM_DTYPE)
        in_maps = _prep_fft(x, W, D, bias)
    else:
        nc = _build_dense(MM_DTYPE)
        in_maps = _prep_dense(x, W, D, bias, MM_DTYPE)

    res = run_bass_kernel_spmd(nc, in_maps, list(range(NCORES)), trace=trace)
    out = np.empty((BATCH, D_OUT), dtype=np.float32)
    for c in range(NCORES):
        oT = res.results[c]["outT"]                  # [i, t, b]
        out[c * BC : (c + 1) * BC, :] = oT.transpose(2, 0, 1).reshape(BC, D_OUT)
    return out, res


def kernel(**inputs) -> np.ndarray:
    out, _ = _run(inputs, trace=False)
    return out


# revision 11
# speedup vs baseline: 1.1687x; 1.0722x over previous
"""Block-circulant matmul kernel for Trainium2 (8 NeuronCores, data-parallel).

Computes out = (x * D) @ M + bias where M is the 4096x4096 block-circulant
matrix built from W[32, 32, 128] (block (i,j) is C_ij[s,t] = W[i,j,(s-t)%128]).

Sharding: batch (4096) split 8 ways -> 512 rows per core; weights replicated.

Two implementations:
 - "fft": 3-stage frequency-domain factorization. Per core: DFT-as-matmul
   (32 mm) -> DVE 32x32 stream-transpose -> per-frequency-slot block-diag
   matmul (32 mm) -> DVE transpose -> iDFT-as-matmul (32 mm). The sigma
   frequency packing puts the 4 real components of a frequency pair-slot c
   at spectrum positions {c, 32+c, 64+c, 96+c} so the quadrant-local DVE
   transpose lands rows exactly where the next stage's matmul needs them.
 - "dense": single big GEMM against the host-materialized circulant matrix.

Everything device-side computes out^T: feature dims on SBUF partitions,
batch on the free dimension.
"""

import os
import numpy as np

import concourse.bass as bass
import concourse.mybir as mybir
from concourse import bacc
from concourse.tile import TileContext
from concourse.bass_utils import run_bass_kernel_spmd

# Problem constants (hardcoded per harness contract).
BATCH = 4096
D_IN = 4096
D_OUT = 4096
BS = 128          # circulant block size
KI = 32           # input blocks
KO = 32           # output blocks
NCORES = 8
BC = BATCH // NCORES      # 512 batch rows per core
NSPLIT = 2                # batch halves per core (pipeline + PSUM sizing)
BH = BC // NSPLIT

IMPL = os.environ.get("BC_IMPL", "fft")
MM_DTYPE = os.environ.get("BC_DTYPE", "fp32")

_NC_CACHE = {}
_PACK_CACHE = {}


def _dt_of(name):
    return {
        "fp32": mybir.dt.float32,
        "f32r": mybir.dt.float32r,
        "bf16": mybir.dt.bfloat16,
    }[name]


# ---------------------------------------------------------------- sigma pack
def _sigma_components():
    """slot c, quadrant Q -> ("re"|"im", f). Pairs (2c+1, 2c+2) for c<31,
    slot 31 holds (63 complex, 0 real, 64 real)."""
    comp = {}
    for c in range(32):
        fa = 2 * c + 1 if c < 31 else 63
        comp[(0, c)] = ("re", fa)
        comp[(1, c)] = ("im", fa)
        if c < 31:
            comp[(2, c)] = ("re", 2 * c + 2)
            comp[(3, c)] = ("im", 2 * c + 2)
        else:
            comp[(2, c)] = ("re", 0)
            comp[(3, c)] = ("re", 64)
    return comp


def _pack_const():
    """Input-independent factor matrices Csig [s, m] and Esig [m, t]."""
    if "const" in _PACK_CACHE:
        return _PACK_CACHE["const"]
    comp = _sigma_components()
    s = np.arange(BS)
    Csig = np.zeros((BS, 128), dtype=np.float64)
    Esig = np.zeros((128, BS), dtype=np.float64)
    for (Q, c), (typ, f) in comp.items():
        m = 32 * Q + c
        ang = 2 * np.pi * f * s / BS
        a = (1.0 if f in (0, 64) else 2.0) / BS
        if typ == "re":
            Csig[:, m] = np.cos(ang)
            Esig[m, :] = a * np.cos(ang)
        else:
            Csig[:, m] = -np.sin(ang)
            Esig[m, :] = -a * np.sin(ang)
    out = (Csig.astype(np.float32), np.ascontiguousarray(Esig.astype(np.float32)))
    _PACK_CACHE["const"] = out
    return out


def _pack_wb(W):
    """Frequency-domain block-diagonal weights WBt [row=(Qr,j), slot, col=(Qc,i)]."""
    comp = _sigma_components()
    Wf = np.fft.fft(W.astype(np.float64), axis=-1)
    Wfr, Wfi = Wf.real, Wf.imag
    WB = np.zeros((32, 128, 128), dtype=np.float64)
    for c in range(32):
        for (qre, qim) in ((0, 1), (2, 3)):
            typ_im = comp[(qim, c)][0]
            f = comp[(qre, c)][1]
            if typ_im == "im":
                wr = Wfr[:, :, f].T  # [j, i]
                wi = Wfi[:, :, f].T
                WB[c, qre*32:(qre+1)*32, qre*32:(qre+1)*32] = wr
                WB[c, qim*32:(qim+1)*32, qre*32:(qre+1)*32] = wi
                WB[c, qre*32:(qre+1)*32, qim*32:(qim+1)*32] = -wi
                WB[c, qim*32:(qim+1)*32, qim*32:(qim+1)*32] = wr
            else:
                f2 = comp[(qim, c)][1]
                WB[c, qre*32:(qre+1)*32, qre*32:(qre+1)*32] = Wfr[:, :, f].T
                WB[c, qim*32:(qim+1)*32, qim*32:(qim+1)*32] = Wfr[:, :, f2].T
    return np.ascontiguousarray(
        WB.transpose(1, 0, 2).astype(np.float32)  # [row, slot, col]
    )


# ---------------------------------------------------------------- fft build
def _build_fft(mm_dtype):
    key = ("fft", mm_dtype)
    if key in _NC_CACHE:
        return _NC_CACHE[key]
    DT = _dt_of(mm_dtype)
    f32 = mybir.dt.float32
    need_round = DT == mybir.dt.float32r
    nsplit = 2
    bh = BC // nsplit

    nc = bacc.Bacc(None, target_bir_lowering=False, debug=False)

    xT = nc.dram_tensor("xT", [BS, KI, BC], DT, kind="ExternalInput")
    Csig_d = nc.dram_tensor("Csig", [BS, 128], f32, kind="ExternalInput")
    WBt_d = nc.dram_tensor("WBt", [128, 32, 128], DT, kind="ExternalInput")
    Esig_d = nc.dram_tensor("Esig", [128, BS], DT, kind="ExternalInput")
    Dt_d = nc.dram_tensor("Dt", [BS, KI], f32, kind="ExternalInput")
    bT_d = nc.dram_tensor("bT", [BS, KO], f32, kind="ExternalInput")
    outT = nc.dram_tensor("outT", [KO, BS, BC], f32, kind="ExternalOutput")

    def copy_engine(k):
        return nc.vector if (k % 2 == 0) else None  # None -> scalar ACTIVATE

    def do_copy(k, out, in_):
        if k % 2 == 0:
            nc.vector.tensor_copy(out=out, in_=in_)
        else:
            nc.scalar.activation(
                out=out, in_=in_, func=mybir.ActivationFunctionType.Copy
            )

    with TileContext(nc) as tc:
        with tc.tile_pool(name="consts", bufs=1) as cpool, \
             tc.tile_pool(name="stage", bufs=4) as spool, \
             tc.tile_pool(name="big1", bufs=2) as big1, \
             tc.tile_pool(name="big2", bufs=2) as big2, \
             tc.tile_pool(name="big3", bufs=1) as big3, \
             tc.tile_pool(name="o", bufs=4) as opool, \
             tc.tile_pool(name="psAll", bufs=8, space="PSUM") as psAll:

            psA = psB = psC = psAll
            csig = cpool.tile([BS, 128], f32)
            esig = cpool.tile([128, BS], DT)
            dt_t = cpool.tile([BS, KI], f32)
            bt_t = cpool.tile([BS, KO], f32)
            wb = cpool.tile([128, 32, 128], DT)
            cd = cpool.tile([BS, KI, 128], DT)
            nc.sync.dma_start(out=csig, in_=Csig_d[:, :])
            nc.sync.dma_start(out=esig, in_=Esig_d[:, :])
            nc.sync.dma_start(out=dt_t, in_=Dt_d[:, :])
            nc.sync.dma_start(out=bt_t, in_=bT_d[:, :])
            nc.sync.dma_start(out=wb, in_=WBt_d[:, :, :])

            # Fold the Bernoulli diagonal into per-j DFT weights.
            for j in range(KI):
                nc.vector.tensor_scalar_mul(
                    out=cd[:, j, :], in0=csig, scalar1=dt_t[:, j : j + 1]
                )

            tdt = f32 if need_round else DT  # transpose-side dtype

            # ---- stage A: spectrum XF[m, b, j] (j innermost for T1 chunks)
            # h-outer so xf[0] completes early and T1(0) overlaps A(h=1).
            xf = [big1.tile([128, bh, KI], tdt, tag="big1", name=f"xf{h}")
                  for h in range(nsplit)]
            for h in range(nsplit):
                for j in range(KI):
                    st = spool.tile([BS, bh], DT, tag="stage")
                    nc.sync.dma_start(
                        out=st, in_=xT[:, j, h * bh : (h + 1) * bh]
                    )
                    ps = psA.tile([128, bh], f32, tag="ps", name=f"psa{j}_{h}")
                    nc.tensor.matmul(ps, cd[:, j, :], st, start=True, stop=True)
                    do_copy(j, xf[h][:, :, j], ps)

            for h in range(nsplit):
                # ---- T1: Z[(Q,j), b, c] = XF[(Q,c), b, j]
                z = big2.tile([128, 32, bh], tdt, tag="big2", name=f"z{h}")
                nc.vector.transpose(out=z.transpose([0, 2, 1]), in_=xf[h])
                if need_round:
                    zr = big3.tile([128, 32, bh], DT, tag="big3", name=f"zr{h}")
                    nc.vector.tensor_copy(out=zr, in_=z)
                    z = zr
                # ---- stage B: per-slot block-diagonal frequency matmul
                yz = big1.tile([128, bh, 32], tdt, tag="big1", name=f"yz{h}")
                for c in range(32):
                    ps = psB.tile([128, bh], f32, tag="ps", name=f"psb{c}_{h}")
                    nc.tensor.matmul(
                        ps, wb[:, c, :], z[:, c, :], start=True, stop=True
                    )
                    do_copy(c, yz[:, :, c], ps)
                # ---- T2: YW[(Q,c), b, i] = YZ[(Q,i), b, c]
                yw = big2.tile([128, 32, bh], tdt, tag="big2", name=f"yw{h}")
                nc.vector.transpose(out=yw.transpose([0, 2, 1]), in_=yz)
                if need_round:
                    ywr = big3.tile([128, 32, bh], DT, tag="big3", name=f"ywr{h}")
                    nc.vector.tensor_copy(out=ywr, in_=yw)
                    yw = ywr
                # ---- stage C: iDFT + bias
                for i in range(KO):
                    ps = psC.tile([128, bh], f32, tag="ps", name=f"psc{i}_{h}")
                    nc.tensor.matmul(
                        ps, esig, yw[:, i, :], start=True, stop=True
                    )
                    oi = opool.tile([BS, bh], f32, tag="o")
                    nc.scalar.activation(
                        out=oi, in_=ps,
                        func=mybir.ActivationFunctionType.Identity,
                        bias=bt_t[:, i : i + 1],
                    )
                    nc.sync.dma_start(
                        out=outT[i, :, h * bh : (h + 1) * bh], in_=oi
                    )

    nc.compile()
    _NC_CACHE[key] = nc
    return nc


def _prep_fft(x, W, D, bias):
    Csig, Esig = _pack_const()
    WBt = _pack_wb(W)
    Dt = np.ascontiguousarray(D.reshape(KI, BS).T)
    bT = np.ascontiguousarray(bias.reshape(KO, BS).T)
    in_maps = []
    for c in range(NCORES):
        xs = x[c * BC : (c + 1) * BC, :]
        xTc = np.ascontiguousarray(xs.reshape(BC, KI, BS).transpose(2, 1, 0))
        in_maps.append(
            {"xT": xTc, "Csig": Csig, "WBt": WBt, "Esig": Esig, "Dt": Dt, "bT": bT}
        )
    return in_maps


# --------------------------------------------------------------- dense build
def _build_dense(mm_dtype):
    key = ("dense", mm_dtype)
    if key in _NC_CACHE:
        return _NC_CACHE[key]
    wdt = _dt_of(mm_dtype)
    f32 = mybir.dt.float32

    nc = bacc.Bacc(None, target_bir_lowering=False, debug=False)

    xT = nc.dram_tensor("xT", [BS, KI, BC], f32, kind="ExternalInput")
    WT = nc.dram_tensor("WT", [KO, BS, KI, BS], wdt, kind="ExternalInput")
    Dt = nc.dram_tensor("Dt", [BS, KI], f32, kind="ExternalInput")
    bT = nc.dram_tensor("bT", [BS, KO], f32, kind="ExternalInput")
    outT = nc.dram_tensor("outT", [KO, BS, BC], f32, kind="ExternalOutput")

    xd_dt = f32 if mm_dtype == "fp32" else wdt

    with TileContext(nc) as tc:
        with tc.tile_pool(name="consts", bufs=1) as cpool, \
             tc.tile_pool(name="stage", bufs=6) as spool, \
             tc.tile_pool(name="xd", bufs=1) as xdpool, \
             tc.tile_pool(name="w", bufs=3) as wpool, \
             tc.tile_pool(name="o", bufs=3) as opool, \
             tc.tile_pool(name="ps", bufs=4, space="PSUM") as pspool:

            dt_tile = cpool.tile([BS, KI], f32)
            bt_tile = cpool.tile([BS, KO], f32)
            nc.sync.dma_start(out=dt_tile, in_=Dt[:, :])
            nc.sync.dma_start(out=bt_tile, in_=bT[:, :])

            xd = xdpool.tile([BS, KI, BC], xd_dt)
            for j in range(KI):
                st = spool.tile([BS, BC], f32, tag="stage")
                nc.sync.dma_start(out=st, in_=xT[:, j, :])
                nc.vector.tensor_scalar_mul(
                    out=xd[:, j, :], in0=st, scalar1=dt_tile[:, j : j + 1]
                )

            for i in range(KO):
                wi = wpool.tile([BS, KI, BS], wdt, tag="w")
                nc.sync.dma_start(out=wi, in_=WT[i])
                ps = pspool.tile([BS, BC], f32, tag="ps")
                for j in range(KI):
                    nc.tensor.matmul(
                        ps, wi[:, j, :], xd[:, j, :],
                        start=(j == 0), stop=(j == KI - 1),
                    )
                oi = opool.tile([BS, BC], f32, tag="o")
                nc.vector.tensor_scalar_add(
                    out=oi, in0=ps, scalar1=bt_tile[:, i : i + 1]
                )
                nc.sync.dma_start(out=outT[i], in_=oi)

    nc.compile()
    _NC_CACHE[key] = nc
    return nc


def _prep_dense(x, W, D, bias, mm_dtype):
    s = np.arange(BS)
    roll = (s[:, None] - s[None, :]) % BS
    M4 = W[:, :, roll]                                   # [i, j, s, t]
    WT = np.ascontiguousarray(M4.transpose(0, 2, 1, 3))  # [i, s, j, t]
    if mm_dtype == "bf16":
        import ml_dtypes
        WT = WT.astype(ml_dtypes.bfloat16)
    Dt = np.ascontiguousarray(D.reshape(KI, BS).T)
    bT = np.ascontiguousarray(bias.reshape(KO, BS).T)
    in_maps = []
    for c in range(NCORES):
        xs = x[c * BC : (c + 1) * BC, :]
        xTc = np.ascontiguousarray(xs.reshape(BC, KI, BS).transpose(2, 1, 0))
        in_maps.append({"xT": xTc, "WT": WT, "Dt": Dt, "bT": bT})
    return in_maps


# ------------------------------------------------------------------- driver
def _run(inputs, trace=False):
    x = np.asarray(inputs["x"], dtype=np.float32)
    W = np.asarray(inputs["W"], dtype=np.float32)
    D = np.asarray(inputs["D_bernoulli"], dtype=np.float32)
    bias = np.asarray(inputs["bias"], dtype=np.float32)

    if IMPL == "fft":
        nc = _build_fft(MM_DTYPE)
        in_maps = _prep_fft(x, W, D, bias)
    else:
        nc = _build_dense(MM_DTYPE)
        in_maps = _prep_dense(x, W, D, bias, MM_DTYPE)

    res = run_bass_kernel_spmd(nc, in_maps, list(range(NCORES)), trace=trace)
    out = np.empty((BATCH, D_OUT), dtype=np.float32)
    for c in range(NCORES):
        oT = res.results[c]["outT"]                  # [i, t, b]
        out[c * BC : (c + 1) * BC, :] = oT.transpose(2, 0, 1).reshape(BC, D_OUT)
    return out, res


def kernel(**inputs) -> np.ndarray:
    out, _ = _run(inputs, trace=False)
    return out
